# revision 1
# baseline (speedup 1.0000x reference)
"""ChainCRF negative-log-likelihood kernel for 8 Trainium2 NeuronCores.

Strategy
--------
The heavy part of the reference is the forward (alpha) recursion
    fv_t[b,j] = logsumexp_i(fv_{t-1}[b,i] + A[i,j]) + feat[b,t,j]
run for T=256 steps over a 128-tag chain, batch 256.

We run it in exp-space:  q_t = (E^T q_{t-1}) * ef_t  with E = exp(A) and
ef_t[j,b] = exp(feat[b,t,j]) / s_tb  (host-prescaled so every column of
ef sums to 1; the log of the prescale is added back on the host).  That
makes the device inner loop exactly one bf16 matmul (tags on the PSUM
partition axis, batch on the free axis, fp32 PSUM accumulate) plus one
elementwise multiply per time step — no per-step transposes and no
per-step normalisation.  The fp32 emission factors are applied by the
DVE, so the only bf16 roundings are the fixed E matrix and the q state.

Every 32 steps a colsum renormalisation keeps the bf16/fp32 range: a
ones-vector matmul reduces q to colsums, the DVE takes reciprocals, a
rank-1 matmul broadcasts them, and — because scaling commutes with the
linear recursion — the scale is applied LAG steps later, keeping all of
the renorm work except one fused multiply off the critical path.  The
applied (bf16-exact) reciprocals are written back to HBM and their logs
are added on the host.

Sharding: data-parallel over batch. Batch indices are sorted by sequence
length (desc) and dealt round-robin to the 8 cores, so all cores see an
identical *shared* active-column profile act_t = #(slot-min lengths > t);
the compiled program simply shrinks the matmul free dim as sequences
finish — masking costs zero instructions.  Each slot runs on device for
min-over-cores(length) steps; the handful of leftover per-column steps
(slot-min vs true length) are finished on the host in float64, which is
exact and ~1k tiny matvecs in numpy.

The gold-path score is pure gather/sum over the inputs and is computed
on the host in float64.
"""

import sys

for _p in (
    "/opt/trn_rl_repo",
    "/root/.axon_site/_ro/trn_rl_repo",
    "/root/.axon_site/_ro/pypackages",
    "/root/.axon_site",
):
    if _p not in sys.path:
        sys.path.append(_p)

import numpy as np
import ml_dtypes

import concourse.bass as bass
import concourse.bacc as bacc
import concourse.tile as tile
from concourse import mybir
from concourse.bass_utils import run_bass_kernel_spmd

N_TAGS = 128
ROOT = 126
END = 127
NCORES = 8
NB = 32          # batch columns per core
RENORM = 32      # device renormalisation cadence (steps)
LAG = 4          # renorm scale applied this many steps after measuring
CHUNK = 32       # ef DMA chunk, in time steps
CHUNK0 = 8       # first (small) chunk so compute starts early

_last_results = None      # BassKernelResults of the most recent device run
_last_nc = None           # program of the most recent device run
_last_in_maps = None      # per-core inputs of the most recent device run
_program_cache = {}       # act_profile tuple -> Bass program


def benchmark(n=3):
    """Re-run the last device launch n times; returns wall seconds each."""
    import time as _time

    out = []
    for _ in range(n):
        t0 = _time.time()
        run_bass_kernel_spmd(_last_nc, _last_in_maps, list(range(NCORES)))
        out.append(_time.time() - t0)
    return out


def _chunk_bounds(Tdev):
    """[(start_t, end_t)] DMA chunks of the ef stream."""
    bounds = [(0, min(CHUNK0, Tdev))]
    t = CHUNK0
    while t < Tdev:
        bounds.append((t, min(t + CHUNK, Tdev)))
        t += CHUNK
    return bounds


def _renorm_plan(act_profile):
    """[(measure_t, apply_t)] with apply inside the loop and nonempty."""
    Tdev = len(act_profile)
    plan = []
    for t in range(RENORM, Tdev, RENORM):
        ta = t + LAG
        if ta < Tdev and act_profile[ta] > 0 and act_profile[t] > 0:
            plan.append((t, ta))
    return plan


NGROUPS = 2      # interleaved column groups (overlaps engine access latencies)


def _build_program(act_profile, ngroups=NGROUPS):
    """One SPMD program shared by all 8 cores.

    act_profile[t] (t = 1..Tdev-1) is the number of active batch columns
    at step t; it is non-increasing and act_profile[1] > 0.
    """
    Tdev = len(act_profile)  # includes t=0 slot (act_profile[0] unused)
    f32 = mybir.dt.float32
    bf16 = mybir.dt.bfloat16
    plan = _renorm_plan(act_profile)
    nren = max(1, len(plan))
    measure = {t: ri for ri, (t, _) in enumerate(plan)}
    apply_at = {ta: ri for ri, (_, ta) in enumerate(plan)}
    bounds = _chunk_bounds(Tdev)
    gw = NB // ngroups  # group width

    def gslices(act):
        """[(lo, hi)] nonempty per-group column ranges covering [0, act)."""
        out = []
        for g in range(ngroups):
            lo, hi = g * gw, min((g + 1) * gw, act)
            if hi > lo:
                out.append((lo, hi))
        return out

    nc = bacc.Bacc("TRN2", debug=False, num_devices=NCORES)
    e_d = nc.dram_tensor("emat", [N_TAGS, N_TAGS], bf16, kind="ExternalInput")
    ef_d = nc.dram_tensor("ef", [N_TAGS, Tdev * NB], f32, kind="ExternalInput")
    qout_d = nc.dram_tensor("q_out", [N_TAGS, NB], bf16, kind="ExternalOutput")
    rout_d = nc.dram_tensor("r_out", [1, nren * NB], bf16, kind="ExternalOutput")

    with tile.TileContext(nc) as tc:
        with (
            tc.tile_pool(name="const", bufs=1) as const_pool,
            tc.tile_pool(name="efp", bufs=1) as ef_pool,
            tc.tile_pool(name="state", bufs=1) as state_pool,
            tc.tile_pool(name="pmm", bufs=2, space="PSUM") as pmm_pool,
            tc.tile_pool(name="pnrm", bufs=2, space="PSUM") as pnrm_pool,
            tc.tile_pool(name="pbc", bufs=2, space="PSUM") as pbc_pool,
        ):
            e_t = const_pool.tile([N_TAGS, N_TAGS], bf16, tag="emat")
            nc.sync.dma_start(e_t[:], e_d[:])
            ones_col = const_pool.tile([N_TAGS, 1], bf16, tag="ones_col")
            nc.vector.memset(ones_col[:], 1.0)
            ones_row = const_pool.tile([1, N_TAGS], bf16, tag="ones_row")
            nc.vector.memset(ones_row[:], 1.0)

            q = state_pool.tile([N_TAGS, NB], bf16, tag="q")
            rstore = state_pool.tile([1, nren * NB], bf16, tag="rstore")
            nc.vector.memset(rstore[:], 1.0)
            rscratch = state_pool.tile([1, NB], f32, tag="rscratch")

            ef_tiles = []
            for (t0, t1) in bounds:
                et = ef_pool.tile([N_TAGS, (t1 - t0) * NB], f32, tag=f"ef{t0}")
                nc.sync.dma_start(et[:], ef_d[:, t0 * NB : t1 * NB])
                ef_tiles.append(et)

            def ef_slice(t, width):
                for (t0, t1), et in zip(bounds, ef_tiles):
                    if t0 <= t < t1:
                        return et[:, (t - t0) * NB : (t - t0) * NB + width]
                raise AssertionError(t)

            # init q (bf16) from the fp32 ef_0
            nc.vector.tensor_copy(q[:], ef_slice(0, NB))

            bc_tiles = [None] * nren
            for t in range(1, Tdev):
                act = act_profile[t]
                if act == 0:
                    break
                mms = []
                for (lo, hi) in gslices(act):
                    mm = pmm_pool.tile([N_TAGS, gw], f32, tag=f"mm{lo}")
                    nc.tensor.matmul(
                        mm[:, : hi - lo], e_t[:, :], q[:, lo:hi],
                        start=True, stop=True,
                    )
                    mms.append(mm)
                for mm, (lo, hi) in zip(mms, gslices(act)):
                    nc.vector.tensor_mul(
                        q[:, lo:hi], mm[:, : hi - lo],
                        ef_slice(t, act)[:, lo:hi],
                    )

                if t in apply_at:
                    ri = apply_at[t]
                    nc.vector.tensor_mul(
                        q[:, :act], q[:, :act], bc_tiles[ri][:, :act]
                    )

                if t in measure:
                    ri = measure[t]
                    a_ap = act_profile[plan[ri][1]]  # width needed at apply
                    cs = pnrm_pool.tile([1, NB], f32, tag="cs")
                    nc.tensor.matmul(
                        cs[:1, :act], ones_col[:, :], q[:, :act],
                        start=True, stop=True,
                    )
                    nc.vector.reciprocal(rscratch[:1, :act], cs[:1, :act])
                    rslice = rstore[:1, ri * NB : ri * NB + act]
                    nc.vector.tensor_copy(rslice, rscratch[:1, :act])
                    bc = pbc_pool.tile([N_TAGS, NB], f32, tag="bc")
                    nc.tensor.matmul(
                        bc[:, :a_ap], ones_row[:1, :],
                        rstore[:1, ri * NB : ri * NB + a_ap],
                        start=True, stop=True,
                    )
                    bc_tiles[ri] = bc

            nc.sync.dma_start(qout_d[:], q[:])
            nc.sync.dma_start(rout_d[:], rstore[:])

    nc.finalize()
    return nc


def kernel(feats, tags, mask, log_transitions):
    global _last_results, _last_nc, _last_in_maps
    feats = np.asarray(feats, dtype=np.float32)
    tags = np.asarray(tags)
    mask = np.asarray(mask)
    lt = np.asarray(log_transitions, dtype=np.float32)
    bsz, T, n = feats.shape
    assert (bsz, T, n) == (256, 256, N_TAGS)

    lengths = mask.astype(np.int64).sum(1)
    order = np.argsort(-lengths, kind="stable")  # desc
    lmin = lengths[order[7::8]]                  # slot-min profile, len NB
    Tdev = max(int(lmin[0]), 2)
    act_profile = [int((lmin > t).sum()) for t in range(Tdev)]
    plan = _renorm_plan(act_profile)

    E64 = np.exp(lt.astype(np.float64))
    Ebf = E64.astype(np.float32).astype(ml_dtypes.bfloat16)
    Eend64 = E64[:, END]

    # --- per-core host preprocessing ---
    feats64 = feats.astype(np.float64)
    in_maps = []
    corr_all = np.zeros((NCORES, NB))
    idx_all = np.zeros((NCORES, NB), np.int64)
    ef0_all = np.zeros((NCORES, N_TAGS, NB), np.float64)
    for c in range(NCORES):
        idx = order[c::8][:NB]
        idx_all[c] = idx
        f = feats64[idx, :Tdev, :]               # [NB, Tdev, 128]
        ef = np.exp(f)
        ef[:, 0, :] *= np.exp(lt[ROOT].astype(np.float64))[None, :]
        s = ef.sum(axis=2)                       # [NB, Tdev]
        ef /= s[:, :, None]
        ef0_all[c] = ef[:, 0, :].T
        # correction: device applies steps t=0..lmin_k-1 for slot k
        tgrid = np.arange(Tdev)[None, :]                 # [1, Tdev]
        corr_all[c] = (np.log(s) * (tgrid < lmin[:, None])).sum(axis=1)
        efc = np.ascontiguousarray(
            ef.transpose(2, 1, 0), dtype=np.float32
        ).reshape(N_TAGS, Tdev * NB)
        in_maps.append({"emat": Ebf, "ef": efc})

    key = tuple(act_profile)
    if key not in _program_cache:
        _program_cache[key] = _build_program(act_profile)
    nc = _program_cache[key]

    _last_nc, _last_in_maps = nc, in_maps
    res = run_bass_kernel_spmd(nc, in_maps, list(range(NCORES)))
    _last_results = res

    # --- host fixup + assembly (float64) ---
    partition = np.zeros(bsz)
    for c in range(NCORES):
        qf = res.results[c]["q_out"].astype(np.float64)          # [128, NB]
        rv = res.results[c]["r_out"].reshape(-1, NB).astype(np.float64)
        # scale rv[ri, k] was applied to slot k at step plan[ri][1]
        # iff k < act_profile[plan[ri][1]]
        off = np.zeros(NB)
        for ri, (tm, ta) in enumerate(plan):
            a = act_profile[ta]
            off[:a] -= np.log(rv[ri, :a])
        for k in range(NB):
            b = idx_all[c, k]
            if lmin[k] < 2:
                q64 = ef0_all[c][:, k].copy()    # device never wrote this slot
                o = 0.0
            else:
                q64 = qf[:, k]
                o = off[k]
            for t in range(int(lmin[k]), int(lengths[b])):
                q64 = (E64.T @ q64) * np.exp(feats64[b, t])
            partition[b] = np.log(Eend64 @ q64) + o + corr_all[c, k]

    # --- gold path score (host, float64) ---
    maskf = mask.astype(np.float64)
    ltd = lt.astype(np.float64)
    trans_tt = ltd[tags[:, :-1], tags[:, 1:]]
    emis = np.take_along_axis(
        feats64[:, :-1, :], tags[:, :-1, None].astype(np.int64), axis=2
    )[..., 0]
    scores = ltd[ROOT, tags[:, 0]]
    scores = scores + (trans_tt * maskf[:, 1:] + emis * maskf[:, :-1]).sum(axis=1)
    last_idx = (maskf.sum(axis=1) - 1.0).astype(np.int64)
    last_tags = np.take_along_axis(np.asarray(tags, np.int64), last_idx[:, None], axis=1)[:, 0]
    last_input = np.take_along_axis(feats64[:, -1, :], last_tags[:, None], axis=1)[:, 0]
    scores = scores + ltd[last_tags, END] + last_input * maskf[:, -1]

    return np.asarray((partition - scores).mean(), dtype=np.float32)



# revision 5
# speedup vs baseline: 1.7994x; 1.7994x over previous
"""ChainCRF negative-log-likelihood kernel for 8 Trainium2 NeuronCores.

Strategy
--------
The heavy part of the reference is the forward (alpha) recursion
    fv_t[b,j] = logsumexp_i(fv_{t-1}[b,i] + A[i,j]) + feat[b,t,j]
run for T=256 steps over a 128-tag chain, batch 256.

We run it in exp-space:  q_t = (E^T q_{t-1}) * ef_t  with E = exp(A) and
ef_t[j,b] = exp(feat[b,t,j]) / s_tb  (host-prescaled so every column of
ef sums to 1; the log of the prescale is added back on the host).  The
device inner loop is one bf16 matmul (tags on the PSUM partition axis,
batch on the free axis) plus one DVE multiply per time step.

The per-step loop latency (~540ns) is fixed hardware latency: PE PSUM
drain, DVE PSUM access, semaphore hops.  Since the recursion is LINEAR
per batch column (q_t = D_t E^T q_{t-1}), we halve the serial depth by
meeting in the middle: a *backward* chain
    g_{t-1} = ef_{t-1} * (E g_t)      (g_t = ef_t * beta_t)
runs concurrently from the sequence end, and the two chains meet at
step m where the host computes  partition = g_{m-1}^T E^T q_{m-2}
in float64.  Both chains have the identical matmul->multiply shape, so
they interleave on the PE/DVE queues and the wall time is
~max(m-2, Tdev-m) slots instead of Tdev slots.

Every 32 steps each chain gets a colsum renormalisation to hold bf16
range: a ones-vector matmul reduces the state to colsums, the DVE takes
reciprocals, a rank-1 matmul broadcasts them, and — because scaling
commutes with the linear recursion — the scale is applied LAG steps
later, keeping the renorm off the critical path.  Applied reciprocals
are written back to HBM and their logs are added on the host.

Sharding: data-parallel over batch.  Batch indices are sorted by length
(desc) and dealt round-robin to the 8 cores, so all cores share one
active-column profile act[t] = #(slot-min lengths > t): the forward
chain's matmul free dim shrinks as sequences finish, the backward
chain's grows as sequences join (their init vectors are preloaded into
the state tile by DMA, so a join costs zero instructions).  Per-column
leftover steps (slot-min vs true length) run on the host in float64 as
a backward chain over [lmin_k, L_k).

The gold-path score is pure gather/sum over the inputs, done on host.
"""

import sys

for _p in (
    "/opt/trn_rl_repo",
    "/root/.axon_site/_ro/trn_rl_repo",
    "/root/.axon_site/_ro/pypackages",
    "/root/.axon_site",
):
    if _p not in sys.path:
        sys.path.append(_p)

import numpy as np
import ml_dtypes

import concourse.bass as bass
import concourse.bacc as bacc
import concourse.tile as tile
from concourse import mybir
from concourse.bass_utils import run_bass_kernel_spmd

N_TAGS = 128
ROOT = 126
END = 127
NCORES = 8
NB = 32          # batch columns per core
RENORM = 32      # device renormalisation cadence (steps)
LAG = 4          # renorm scale applied this many steps after measuring
CHUNK = 32       # ef DMA chunk, in time steps
CHUNK0 = 8       # first (small) chunk per direction so compute starts early

_last_results = None      # BassKernelResults of the most recent device run
_last_nc = None           # program of the most recent device run
_last_in_maps = None      # per-core inputs of the most recent device run
_program_cache = {}       # (act_profile, m) -> Bass program


def benchmark(n=3):
    """Re-run the last device launch n times; returns wall seconds each."""
    import time as _time

    out = []
    for _ in range(n):
        t0 = _time.time()
        run_bass_kernel_spmd(_last_nc, _last_in_maps, list(range(NCORES)))
        out.append(_time.time() - t0)
    return out


def _split_mid(Tdev):
    """Meeting step m: fwd covers t=1..m-2, bwd covers t=Tdev-1..m."""
    if Tdev < 16:
        return Tdev            # bwd empty; short-column host path handles all
    return (Tdev + 2) // 2


def _chunk_bounds(Tdev, m):
    """Interleaved [(start_t, end_t)] DMA chunks: fwd side ascending from 0,
    bwd side descending from Tdev, so both chains' streams arrive in
    consumption order."""
    cut = min(max(m - 1, 0), Tdev)
    fb = []
    t = 0
    step = CHUNK0
    while t < cut:
        fb.append((t, min(t + step, cut)))
        t += step
        step = CHUNK
    bb = []
    t = Tdev
    step = CHUNK0
    while t > cut:
        bb.append((max(t - step, cut), t))
        t -= step
        step = CHUNK
    out = []
    for i in range(max(len(fb), len(bb))):
        if i < len(fb):
            out.append(fb[i])
        if i < len(bb):
            out.append(bb[i])
    return out


def _renorm_plans(act_profile, m):
    """([(measure_t, apply_t)] fwd, [(measure_tb, apply_tb)] bwd)."""
    Tdev = len(act_profile)
    plan_f = []
    for t in range(RENORM, max(m - 1, 0), RENORM):
        ta = t + LAG
        if ta <= m - 2 and act_profile[ta] > 0 and act_profile[t] > 0:
            plan_f.append((t, ta))
    plan_b = []
    for s in range(RENORM, max(Tdev - m + 1, 0), RENORM):
        tb = Tdev - s
        tba = tb - LAG
        if tba >= m and act_profile[tb] > 0:
            plan_b.append((tb, tba))
    return plan_f, plan_b


def _build_program(act_profile, m):
    """One SPMD program shared by all 8 cores.

    act_profile[t] = number of batch columns with slot-min length > t;
    non-increasing, act_profile[1] > 0.  Forward chain runs t = 1..m-2,
    backward chain runs t_b = Tdev-1..m (consuming ef index t_b - 1).
    """
    Tdev = len(act_profile)
    f32 = mybir.dt.float32
    bf16 = mybir.dt.bfloat16
    plan_f, plan_b = _renorm_plans(act_profile, m)
    nrf = max(1, len(plan_f))
    nrb = max(1, len(plan_b))
    measure_f = {t: ri for ri, (t, _) in enumerate(plan_f)}
    apply_f = {ta: ri for ri, (_, ta) in enumerate(plan_f)}
    measure_b = {t: ri for ri, (t, _) in enumerate(plan_b)}
    apply_b = {ta: ri for ri, (_, ta) in enumerate(plan_b)}
    bounds = _chunk_bounds(Tdev, m)

    nc = bacc.Bacc("TRN2", debug=False, num_devices=NCORES)
    ef_d = nc.dram_tensor("emat", [N_TAGS, N_TAGS], bf16, kind="ExternalInput")
    eb_d = nc.dram_tensor("ematT", [N_TAGS, N_TAGS], bf16, kind="ExternalInput")
    efs_d = nc.dram_tensor("ef", [N_TAGS, Tdev * NB], f32, kind="ExternalInput")
    q0_d = nc.dram_tensor("q0", [N_TAGS, NB], bf16, kind="ExternalInput")
    g0_d = nc.dram_tensor("g0", [N_TAGS, NB], bf16, kind="ExternalInput")
    qout_d = nc.dram_tensor("q_out", [N_TAGS, NB], bf16, kind="ExternalOutput")
    gout_d = nc.dram_tensor("g_out", [N_TAGS, NB], bf16, kind="ExternalOutput")
    rf_d = nc.dram_tensor("rf_out", [1, nrf * NB], bf16, kind="ExternalOutput")
    rb_d = nc.dram_tensor("rb_out", [1, nrb * NB], bf16, kind="ExternalOutput")

    with tile.TileContext(nc) as tc:
        with (
            tc.tile_pool(name="const", bufs=1) as const_pool,
            tc.tile_pool(name="efp", bufs=1) as ef_pool,
            tc.tile_pool(name="state", bufs=1) as state_pool,
            tc.tile_pool(name="pmmf", bufs=2, space="PSUM") as pmmf_pool,
            tc.tile_pool(name="pmmb", bufs=2, space="PSUM") as pmmb_pool,
            tc.tile_pool(name="pnrm", bufs=2, space="PSUM") as pnrm_pool,
            tc.tile_pool(name="pbc", bufs=2, space="PSUM") as pbc_pool,
        ):
            e_f = const_pool.tile([N_TAGS, N_TAGS], bf16, tag="emat")
            nc.sync.dma_start(e_f[:], ef_d[:])
            e_b = const_pool.tile([N_TAGS, N_TAGS], bf16, tag="ematT")
            nc.sync.dma_start(e_b[:], eb_d[:])

            q = state_pool.tile([N_TAGS, NB], bf16, tag="q")
            nc.sync.dma_start(q[:], q0_d[:])
            g = state_pool.tile([N_TAGS, NB], bf16, tag="g")
            nc.sync.dma_start(g[:], g0_d[:])

            ones_col = const_pool.tile([N_TAGS, 1], bf16, tag="ones_col")
            nc.vector.memset(ones_col[:], 1.0)
            ones_row = const_pool.tile([1, N_TAGS], bf16, tag="ones_row")
            nc.vector.memset(ones_row[:], 1.0)

            rstore_f = state_pool.tile([1, nrf * NB], bf16, tag="rstore_f")
            nc.vector.memset(rstore_f[:], 1.0)
            rstore_b = state_pool.tile([1, nrb * NB], bf16, tag="rstore_b")
            nc.vector.memset(rstore_b[:], 1.0)
            rscratch = state_pool.tile([1, NB], f32, tag="rscratch")

            ef_tiles = []
            for (t0, t1) in bounds:
                et = ef_pool.tile([N_TAGS, (t1 - t0) * NB], f32, tag=f"ef{t0}")
                nc.sync.dma_start(et[:], efs_d[:, t0 * NB : t1 * NB])
                ef_tiles.append(et)

            def ef_slice(t, width):
                for (t0, t1), et in zip(bounds, ef_tiles):
                    if t0 <= t < t1:
                        return et[:, (t - t0) * NB : (t - t0) * NB + width]
                raise AssertionError(t)

            def renorm_block(t, state, measure, apply_at, plan, rstore,
                             bc_tiles, act):
                """Delayed-scale renormalisation for one chain at step t."""
                if t in apply_at:
                    ri = apply_at[t]
                    nc.vector.tensor_mul(
                        state[:, :act], state[:, :act], bc_tiles[ri][:, :act]
                    )
                if t in measure:
                    ri = measure[t]
                    a_ap = act_profile[plan[ri][1]]  # width needed at apply
                    cs = pnrm_pool.tile([1, NB], f32, tag="cs")
                    nc.tensor.matmul(
                        cs[:1, :act], ones_col[:, :], state[:, :act],
                        start=True, stop=True,
                    )
                    nc.vector.reciprocal(rscratch[:1, :act], cs[:1, :act])
                    rslice = rstore[:1, ri * NB : ri * NB + act]
                    nc.vector.tensor_copy(rslice, rscratch[:1, :act])
                    bc = pbc_pool.tile([N_TAGS, NB], f32, tag="bc")
                    nc.tensor.matmul(
                        bc[:, :a_ap], ones_row[:1, :],
                        rstore[:1, ri * NB : ri * NB + a_ap],
                        start=True, stop=True,
                    )
                    bc_tiles[ri] = bc

            bc_f = [None] * nrf
            bc_b = [None] * nrb
            nslots = max(m - 2, Tdev - m)
            for s in range(1, nslots + 1):
                tf = s                       # forward step index
                tb = Tdev - s                # backward step index
                fon = tf <= m - 2 and act_profile[tf] > 0
                bon = tb >= m and act_profile[tb] > 0
                af = act_profile[tf] if fon else 0
                ab = act_profile[tb] if bon else 0

                if fon:
                    mmf = pmmf_pool.tile([N_TAGS, NB], f32, tag="mmf")
                    nc.tensor.matmul(
                        mmf[:, :af], e_f[:, :], q[:, :af],
                        start=True, stop=True,
                    )
                if bon:
                    mmb = pmmb_pool.tile([N_TAGS, NB], f32, tag="mmb")
                    nc.tensor.matmul(
                        mmb[:, :ab], e_b[:, :], g[:, :ab],
                        start=True, stop=True,
                    )
                if fon:
                    nc.vector.tensor_mul(
                        q[:, :af], mmf[:, :af], ef_slice(tf, af)
                    )
                if bon:
                    nc.vector.tensor_mul(
                        g[:, :ab], mmb[:, :ab], ef_slice(tb - 1, ab)
                    )

                if fon:
                    renorm_block(tf, q, measure_f, apply_f, plan_f,
                                 rstore_f, bc_f, af)
                if bon:
                    renorm_block(tb, g, measure_b, apply_b, plan_b,
                                 rstore_b, bc_b, ab)

            nc.sync.dma_start(qout_d[:], q[:])
            nc.sync.dma_start(gout_d[:], g[:])
            nc.sync.dma_start(rf_d[:], rstore_f[:])
            nc.sync.dma_start(rb_d[:], rstore_b[:])

    nc.finalize()
    return nc


def kernel(feats, tags, mask, log_transitions):
    global _last_results, _last_nc, _last_in_maps
    feats = np.asarray(feats, dtype=np.float32)
    tags = np.asarray(tags)
    mask = np.asarray(mask)
    lt = np.asarray(log_transitions, dtype=np.float32)
    bsz, T, n = feats.shape
    assert (bsz, T, n) == (256, 256, N_TAGS)

    lengths = mask.astype(np.int64).sum(1)
    order = np.argsort(-lengths, kind="stable")  # desc
    lmin = lengths[order[7::8]]                  # slot-min profile, len NB
    Tdev = max(int(lmin[0]), 2)
    act_profile = [int((lmin > t).sum()) for t in range(Tdev)]
    m = _split_mid(Tdev)
    plan_f, plan_b = _renorm_plans(act_profile, m)

    lt64 = lt.astype(np.float64)
    E64 = np.exp(lt64)
    Ebf = E64.astype(np.float32).astype(ml_dtypes.bfloat16)
    EbfT = np.ascontiguousarray(Ebf.T)
    Eend64 = E64[:, END]

    # --- per-core host preprocessing ---
    feats64 = feats.astype(np.float64)
    in_maps = []
    corr_all = np.zeros((NCORES, NB))
    idx_all = np.zeros((NCORES, NB), np.int64)
    ef0_all = np.zeros((NCORES, N_TAGS, NB))
    beta_all = np.zeros((NCORES, NB, N_TAGS))
    logbn_all = np.zeros((NCORES, NB))
    root64 = np.exp(lt64[ROOT])
    for c in range(NCORES):
        idx = order[c::8][:NB]
        idx_all[c] = idx
        f = feats64[idx, :Tdev, :]               # [NB, Tdev, 128]
        ef = np.exp(f)
        ef[:, 0, :] *= root64[None, :]
        s = ef.sum(axis=2)                       # [NB, Tdev]
        ef /= s[:, :, None]
        ef32 = ef.astype(np.float32)             # device values, fp32
        ef0_all[c] = ef32[:, 0, :].T.astype(np.float64)
        # prescale corrections: device consumes indices t < lmin_k
        tgrid = np.arange(Tdev)[None, :]
        corr_all[c] = (np.log(s) * (tgrid < lmin[:, None])).sum(axis=1)

        # host backward chains over [lmin_k, L_k), float64, normalized
        ginit = np.ones((NB, N_TAGS), np.float32)
        for k in range(NB):
            b = idx[k]
            beta = Eend64.copy()
            for t in range(int(lengths[b]) - 1, int(lmin[k]) - 1, -1):
                beta = E64 @ (np.exp(feats64[b, t, :]) * beta)
                sm = beta.sum()
                beta /= sm
                logbn_all[c, k] += np.log(sm)
            beta_all[c, k] = beta
            if lmin[k] >= 1:
                ginit[k] = (
                    ef[k, int(lmin[k]) - 1, :] * beta
                ).astype(np.float32)

        efc = np.ascontiguousarray(
            ef32.transpose(2, 1, 0)
        ).reshape(N_TAGS, Tdev * NB)
        in_maps.append({
            "emat": Ebf,
            "ematT": EbfT,
            "ef": efc,
            "q0": ef32[:, 0, :].T.astype(ml_dtypes.bfloat16),
            "g0": np.ascontiguousarray(ginit.T).astype(ml_dtypes.bfloat16),
        })

    key = (tuple(act_profile), m)
    if key not in _program_cache:
        _program_cache[key] = _build_program(act_profile, m)
    nc = _program_cache[key]

    _last_nc, _last_in_maps = nc, in_maps
    res = run_bass_kernel_spmd(nc, in_maps, list(range(NCORES)))
    _last_results = res

    # --- host assembly (float64) ---
    Ebf64T = Ebf.astype(np.float64).T
    partition = np.zeros(bsz)
    for c in range(NCORES):
        qf = res.results[c]["q_out"].astype(np.float64)          # [128, NB]
        gf = res.results[c]["g_out"].astype(np.float64)          # [128, NB]
        rvf = res.results[c]["rf_out"].reshape(-1, NB).astype(np.float64)
        rvb = res.results[c]["rb_out"].reshape(-1, NB).astype(np.float64)
        off = np.zeros(NB)
        for ri, (tm, ta) in enumerate(plan_f):
            a = act_profile[ta]
            off[:a] -= np.log(rvf[ri, :a])
        for ri, (tm, tba) in enumerate(plan_b):
            a = act_profile[tba]
            off[:a] -= np.log(rvb[ri, :a])
        for k in range(NB):
            b = idx_all[c, k]
            if lmin[k] < 2:
                q64 = ef0_all[c][:, k]
                offk = 0.0
            else:
                q64 = qf[:, k]
                offk = off[k]
            if lmin[k] >= m:
                val = gf[:, k] @ (Ebf64T @ q64)
            else:
                val = beta_all[c, k] @ q64
            partition[b] = (
                np.log(val) + offk + corr_all[c, k] + logbn_all[c, k]
            )

    # --- gold path score (host, float64) ---
    maskf = mask.astype(np.float64)
    trans_tt = lt64[tags[:, :-1], tags[:, 1:]]
    emis = np.take_along_axis(
        feats64[:, :-1, :], tags[:, :-1, None].astype(np.int64), axis=2
    )[..., 0]
    scores = lt64[ROOT, tags[:, 0]]
    scores = scores + (trans_tt * maskf[:, 1:] + emis * maskf[:, :-1]).sum(axis=1)
    last_idx = (maskf.sum(axis=1) - 1.0).astype(np.int64)
    last_tags = np.take_along_axis(np.asarray(tags, np.int64), last_idx[:, None], axis=1)[:, 0]
    last_input = np.take_along_axis(feats64[:, -1, :], last_tags[:, None], axis=1)[:, 0]
    scores = scores + lt64[last_tags, END] + last_input * maskf[:, -1]

    return np.asarray((partition - scores).mean(), dtype=np.float32)


# revision 11
# speedup vs baseline: 1.8241x; 1.0137x over previous
"""ChainCRF negative-log-likelihood kernel for 8 Trainium2 NeuronCores.

Strategy
--------
The heavy part of the reference is the forward (alpha) recursion
    fv_t[b,j] = logsumexp_i(fv_{t-1}[b,i] + A[i,j]) + feat[b,t,j]
run for T=256 steps over a 128-tag chain, batch 256.

We run it in exp-space:  q_t = (E^T q_{t-1}) * ef_t  with E = exp(A) and
ef_t[j,b] = exp(feat[b,t,j]) / s_tb  (host-prescaled so every column of
ef sums to 1; the log of the prescale is added back on the host).  The
device inner loop is one bf16 matmul (tags on the PSUM partition axis,
batch on the free axis) plus one DVE multiply per time step.

The per-step loop latency (~540ns) is fixed hardware latency: PE PSUM
drain, DVE PSUM access, semaphore hops.  Since the recursion is LINEAR
per batch column (q_t = D_t E^T q_{t-1}), we halve the serial depth by
meeting in the middle: a *backward* chain
    g_{t-1} = ef_{t-1} * (E g_t)      (g_t = ef_t * beta_t)
runs concurrently from the sequence end, and the two chains meet at
step m where the host computes  partition = g_{m-1}^T E^T q_{m-2}
in float64.  Both chains have the identical matmul->multiply shape, so
they interleave on the PE/DVE queues and the wall time is
~max(m-2, Tdev-m) slots instead of Tdev slots.

Every 32 steps each chain gets a colsum renormalisation to hold bf16
range: a ones-vector matmul reduces the state to colsums, the DVE takes
reciprocals, a rank-1 matmul broadcasts them, and — because scaling
commutes with the linear recursion — the scale is applied LAG steps
later, keeping the renorm off the critical path.  Applied reciprocals
are written back to HBM and their logs are added on the host.

Sharding: data-parallel over batch.  Batch indices are sorted by length
(desc) and dealt round-robin to the 8 cores, so all cores share one
active-column profile act[t] = #(slot-min lengths > t): the forward
chain's matmul free dim shrinks as sequences finish, the backward
chain's grows as sequences join (their init vectors are preloaded into
the state tile by DMA, so a join costs zero instructions).  Per-column
leftover steps (slot-min vs true length) run on the host in float64 as
a backward chain over [lmin_k, L_k).

The gold-path score is pure gather/sum over the inputs, done on host.
"""

import sys

for _p in (
    "/opt/trn_rl_repo",
    "/root/.axon_site/_ro/trn_rl_repo",
    "/root/.axon_site/_ro/pypackages",
    "/root/.axon_site",
):
    if _p not in sys.path:
        sys.path.append(_p)

import numpy as np
import ml_dtypes

import concourse.bass as bass
import concourse.bacc as bacc
import concourse.tile as tile
from concourse import mybir
from concourse.bass_utils import run_bass_kernel_spmd

N_TAGS = 128
ROOT = 126
END = 127
NCORES = 8
NB = 32          # batch columns per core
RENORM = 32      # device renormalisation cadence (steps)
LAG = 4          # renorm scale applied this many steps after measuring
CHUNK = 32       # ef DMA chunk, in time steps
CHUNK0 = 8       # first (small) chunk per direction so compute starts early

_last_results = None      # BassKernelResults of the most recent device run
_last_nc = None           # program of the most recent device run
_last_in_maps = None      # per-core inputs of the most recent device run
_program_cache = {}       # (act_profile, m) -> Bass program


def benchmark(n=3):
    """Re-run the last device launch n times; returns wall seconds each."""
    import time as _time

    out = []
    for _ in range(n):
        t0 = _time.time()
        run_bass_kernel_spmd(_last_nc, _last_in_maps, list(range(NCORES)))
        out.append(_time.time() - t0)
    return out


def _split_mid(Tdev):
    """Meeting step m: fwd covers t=1..m-2, bwd covers t=Tdev-1..m."""
    if Tdev < 16:
        return Tdev            # bwd empty; short-column host path handles all
    return (Tdev + 2) // 2


def _chunk_bounds(Tdev, m):
    """Interleaved [(start_t, end_t)] DMA chunks: fwd side ascending from 0,
    bwd side descending from Tdev, so both chains' streams arrive in
    consumption order."""
    cut = min(max(m - 1, 0), Tdev)
    fb = []
    t = 0
    step = CHUNK0
    while t < cut:
        fb.append((t, min(t + step, cut)))
        t += step
        step = CHUNK
    bb = []
    t = Tdev
    step = CHUNK0
    while t > cut:
        bb.append((max(t - step, cut), t))
        t -= step
        step = CHUNK
    out = []
    for i in range(max(len(fb), len(bb))):
        if i < len(fb):
            out.append(fb[i])
        if i < len(bb):
            out.append(bb[i])
    return out


def _renorm_plans(act_profile, m):
    """([(measure_t, apply_t)] fwd, [(measure_tb, apply_tb)] bwd)."""
    Tdev = len(act_profile)
    plan_f = []
    for t in range(RENORM, max(m - 1, 0), RENORM):
        ta = t + LAG
        if ta <= m - 2 and act_profile[ta] > 0 and act_profile[t] > 0:
            plan_f.append((t, ta))
    plan_b = []
    for s in range(RENORM, max(Tdev - m + 1, 0), RENORM):
        tb = Tdev - s
        tba = tb - LAG
        if tba >= m and act_profile[tb] > 0:
            plan_b.append((tb, tba))
    return plan_f, plan_b


def _build_program(act_profile, m):
    """One SPMD program shared by all 8 cores.

    act_profile[t] = number of batch columns with slot-min length > t;
    non-increasing, act_profile[1] > 0.  Forward chain runs t = 1..m-2,
    backward chain runs t_b = Tdev-1..m (consuming ef index t_b - 1).
    """
    Tdev = len(act_profile)
    f32 = mybir.dt.float32
    bf16 = mybir.dt.bfloat16
    plan_f, plan_b = _renorm_plans(act_profile, m)
    nrf = max(1, len(plan_f))
    nrb = max(1, len(plan_b))
    measure_f = {t: ri for ri, (t, _) in enumerate(plan_f)}
    apply_f = {ta: ri for ri, (_, ta) in enumerate(plan_f)}
    measure_b = {t: ri for ri, (t, _) in enumerate(plan_b)}
    apply_b = {ta: ri for ri, (_, ta) in enumerate(plan_b)}
    bounds = _chunk_bounds(Tdev, m)

    nc = bacc.Bacc("TRN2", debug=False, num_devices=NCORES)
    ep_d = nc.dram_tensor("epack", [N_TAGS, 2 * N_TAGS], bf16, kind="ExternalInput")
    efs_d = nc.dram_tensor("ef", [N_TAGS, Tdev * NB], f32, kind="ExternalInput")
    g0_d = nc.dram_tensor("g0", [N_TAGS, NB], bf16, kind="ExternalInput")
    qout_d = nc.dram_tensor("q_out", [N_TAGS, NB], bf16, kind="ExternalOutput")
    gout_d = nc.dram_tensor("g_out", [N_TAGS, NB], bf16, kind="ExternalOutput")
    rf_d = nc.dram_tensor("rf_out", [1, nrf * NB], bf16, kind="ExternalOutput")
    rb_d = nc.dram_tensor("rb_out", [1, nrb * NB], bf16, kind="ExternalOutput")

    # last slot whose renorm block touches each rstore: after that the
    # result DMA can be issued mid-loop and overlap the remaining compute
    last_rf = max((t for t, _ in plan_f), default=0)
    last_rb = max((Tdev - t for t, _ in plan_b), default=0)

    with tile.TileContext(nc) as tc:
        with (
            tc.tile_pool(name="const", bufs=1) as const_pool,
            tc.tile_pool(name="efp", bufs=1) as ef_pool,
            tc.tile_pool(name="state", bufs=1) as state_pool,
            tc.tile_pool(name="pmmf", bufs=2, space="PSUM") as pmmf_pool,
            tc.tile_pool(name="pmmb", bufs=2, space="PSUM") as pmmb_pool,
            tc.tile_pool(name="pnrm", bufs=2, space="PSUM") as pnrm_pool,
            tc.tile_pool(name="pbc", bufs=2, space="PSUM") as pbc_pool,
        ):
            epk = const_pool.tile([N_TAGS, 2 * N_TAGS], bf16, tag="epack")
            nc.sync.dma_start(epk[:], ep_d[:])
            e_f = epk[:, 0:N_TAGS]
            e_b = epk[:, N_TAGS : 2 * N_TAGS]

            q = state_pool.tile([N_TAGS, NB], bf16, tag="q")
            g = state_pool.tile([N_TAGS, NB], bf16, tag="g")

            ef_tiles = {}

            def ef_dma(ci):
                t0, t1 = bounds[ci]
                et = ef_pool.tile([N_TAGS, (t1 - t0) * NB], f32, tag=f"ef{t0}")
                nc.sync.dma_start(et[:], efs_d[:, t0 * NB : t1 * NB])
                ef_tiles[ci] = et

            ef_dma(0)                       # first fwd chunk: gates q init
            nc.sync.dma_start(g[:], g0_d[:])
            for ci in range(1, len(bounds)):
                ef_dma(ci)

            ones_col = const_pool.tile([N_TAGS, 1], bf16, tag="ones_col")
            nc.vector.memset(ones_col[:], 1.0)
            ones_row = const_pool.tile([1, N_TAGS], bf16, tag="ones_row")
            nc.vector.memset(ones_row[:], 1.0)

            rstore_f = state_pool.tile([1, nrf * NB], bf16, tag="rstore_f")
            nc.vector.memset(rstore_f[:], 1.0)
            rstore_b = state_pool.tile([1, nrb * NB], bf16, tag="rstore_b")
            nc.vector.memset(rstore_b[:], 1.0)
            rscratch = state_pool.tile([1, NB], f32, tag="rscratch")

            def ef_slice(t, width):
                for ci, (t0, t1) in enumerate(bounds):
                    if t0 <= t < t1:
                        et = ef_tiles[ci]
                        return et[:, (t - t0) * NB : (t - t0) * NB + width]
                raise AssertionError(t)

            # init q (bf16) from the fp32 ef_0
            nc.vector.tensor_copy(q[:], ef_slice(0, NB))

            def renorm_block(t, state, measure, apply_at, plan, rstore,
                             bc_tiles, act):
                """Delayed-scale renormalisation for one chain at step t."""
                if t in apply_at:
                    ri = apply_at[t]
                    nc.vector.tensor_mul(
                        state[:, :act], state[:, :act], bc_tiles[ri][:, :act]
                    )
                if t in measure:
                    ri = measure[t]
                    a_ap = act_profile[plan[ri][1]]  # width needed at apply
                    cs = pnrm_pool.tile([1, NB], f32, tag="cs")
                    nc.tensor.matmul(
                        cs[:1, :act], ones_col[:, :], state[:, :act],
                        start=True, stop=True,
                    )
                    nc.vector.reciprocal(rscratch[:1, :act], cs[:1, :act])
                    # store copy on the Activation engine keeps the in-order
                    # DVE queue from stalling behind one extra renorm scalar
                    rslice = rstore[:1, ri * NB : ri * NB + act]
                    nc.scalar.copy(rslice, rscratch[:1, :act])
                    bc = pbc_pool.tile([N_TAGS, NB], f32, tag="bc")
                    nc.tensor.matmul(
                        bc[:, :a_ap], ones_row[:1, :],
                        rstore[:1, ri * NB : ri * NB + a_ap],
                        start=True, stop=True,
                    )
                    bc_tiles[ri] = bc

            bc_f = [None] * nrf
            bc_b = [None] * nrb
            nslots = max(m - 2, Tdev - m)
            for s in range(1, nslots + 1):
                tf = s                       # forward step index
                tb = Tdev - s                # backward step index
                fon = tf <= m - 2 and act_profile[tf] > 0
                bon = tb >= m and act_profile[tb] > 0
                af = act_profile[tf] if fon else 0
                ab = act_profile[tb] if bon else 0

                if fon:
                    mmf = pmmf_pool.tile([N_TAGS, NB], f32, tag="mmf")
                    nc.tensor.matmul(
                        mmf[:, :af], e_f[:, :], q[:, :af],
                        start=True, stop=True,
                    )
                if bon:
                    mmb = pmmb_pool.tile([N_TAGS, NB], f32, tag="mmb")
                    nc.tensor.matmul(
                        mmb[:, :ab], e_b[:, :], g[:, :ab],
                        start=True, stop=True,
                    )
                if fon:
                    nc.vector.tensor_mul(
                        q[:, :af], mmf[:, :af], ef_slice(tf, af)
                    )
                if bon:
                    nc.vector.tensor_mul(
                        g[:, :ab], mmb[:, :ab], ef_slice(tb - 1, ab)
                    )

                if fon:
                    renorm_block(tf, q, measure_f, apply_f, plan_f,
                                 rstore_f, bc_f, af)
                if bon:
                    renorm_block(tb, g, measure_b, apply_b, plan_b,
                                 rstore_b, bc_b, ab)

                # results DMAs issued mid-loop, right after the last write
                # of each rstore, so they overlap the remaining slots
                if plan_f and s == last_rf + 1:
                    nc.sync.dma_start(rf_d[:], rstore_f[:])
                if plan_b and s == last_rb + 1:
                    nc.sync.dma_start(rb_d[:], rstore_b[:])

            if not plan_f or last_rf + 1 > nslots:
                nc.sync.dma_start(rf_d[:], rstore_f[:])
            if not plan_b or last_rb + 1 > nslots:
                nc.sync.dma_start(rb_d[:], rstore_b[:])
            nc.sync.dma_start(qout_d[:], q[:])
            nc.sync.dma_start(gout_d[:], g[:])

    nc.finalize()
    return nc


def kernel(feats, tags, mask, log_transitions):
    global _last_results, _last_nc, _last_in_maps
    feats = np.asarray(feats, dtype=np.float32)
    tags = np.asarray(tags)
    mask = np.asarray(mask)
    lt = np.asarray(log_transitions, dtype=np.float32)
    bsz, T, n = feats.shape
    assert (bsz, T, n) == (256, 256, N_TAGS)

    lengths = mask.astype(np.int64).sum(1)
    order = np.argsort(-lengths, kind="stable")  # desc
    lmin = lengths[order[7::8]]                  # slot-min profile, len NB
    Tdev = max(int(lmin[0]), 2)
    act_profile = [int((lmin > t).sum()) for t in range(Tdev)]
    m = _split_mid(Tdev)
    plan_f, plan_b = _renorm_plans(act_profile, m)

    lt64 = lt.astype(np.float64)
    E64 = np.exp(lt64)
    Ebf = E64.astype(np.float32).astype(ml_dtypes.bfloat16)
    epack = np.ascontiguousarray(
        np.concatenate([Ebf, Ebf.T], axis=1)
    )
    Eend64 = E64[:, END]

    # --- per-core host preprocessing ---
    feats64 = feats.astype(np.float64)
    in_maps = []
    corr_all = np.zeros((NCORES, NB))
    idx_all = np.zeros((NCORES, NB), np.int64)
    ef0_all = np.zeros((NCORES, N_TAGS, NB))
    beta_all = np.zeros((NCORES, NB, N_TAGS))
    logbn_all = np.zeros((NCORES, NB))
    root64 = np.exp(lt64[ROOT])
    for c in range(NCORES):
        idx = order[c::8][:NB]
        idx_all[c] = idx
        f = feats64[idx, :Tdev, :]               # [NB, Tdev, 128]
        ef = np.exp(f)
        ef[:, 0, :] *= root64[None, :]
        s = ef.sum(axis=2)                       # [NB, Tdev]
        ef /= s[:, :, None]
        ef32 = ef.astype(np.float32)             # device values, fp32
        ef0_all[c] = ef32[:, 0, :].T.astype(np.float64)
        # prescale corrections: device consumes indices t < lmin_k
        tgrid = np.arange(Tdev)[None, :]
        corr_all[c] = (np.log(s) * (tgrid < lmin[:, None])).sum(axis=1)

        # host backward chains over [lmin_k, L_k), float64, normalized
        ginit = np.ones((NB, N_TAGS), np.float32)
        for k in range(NB):
            b = idx[k]
            beta = Eend64.copy()
            for t in range(int(lengths[b]) - 1, int(lmin[k]) - 1, -1):
                beta = E64 @ (np.exp(feats64[b, t, :]) * beta)
                sm = beta.sum()
                beta /= sm
                logbn_all[c, k] += np.log(sm)
            beta_all[c, k] = beta
            if lmin[k] >= 1:
                ginit[k] = (
                    ef[k, int(lmin[k]) - 1, :] * beta
                ).astype(np.float32)

        efc = np.ascontiguousarray(
            ef32.transpose(2, 1, 0)
        ).reshape(N_TAGS, Tdev * NB)
        in_maps.append({
            "epack": epack,
            "ef": efc,
            "g0": np.ascontiguousarray(ginit.T).astype(ml_dtypes.bfloat16),
        })

    key = (tuple(act_profile), m)
    if key not in _program_cache:
        _program_cache[key] = _build_program(act_profile, m)
    nc = _program_cache[key]

    _last_nc, _last_in_maps = nc, in_maps
    res = run_bass_kernel_spmd(nc, in_maps, list(range(NCORES)))
    _last_results = res

    # --- host assembly (float64) ---
    Ebf64T = Ebf.astype(np.float64).T
    partition = np.zeros(bsz)
    for c in range(NCORES):
        qf = res.results[c]["q_out"].astype(np.float64)          # [128, NB]
        gf = res.results[c]["g_out"].astype(np.float64)          # [128, NB]
        rvf = res.results[c]["rf_out"].reshape(-1, NB).astype(np.float64)
        rvb = res.results[c]["rb_out"].reshape(-1, NB).astype(np.float64)
        off = np.zeros(NB)
        for ri, (tm, ta) in enumerate(plan_f):
            a = act_profile[ta]
            off[:a] -= np.log(rvf[ri, :a])
        for ri, (tm, tba) in enumerate(plan_b):
            a = act_profile[tba]
            off[:a] -= np.log(rvb[ri, :a])
        for k in range(NB):
            b = idx_all[c, k]
            if lmin[k] < 2:
                q64 = ef0_all[c][:, k]
                offk = 0.0
            else:
                q64 = qf[:, k]
                offk = off[k]
            if lmin[k] >= m:
                val = gf[:, k] @ (Ebf64T @ q64)
            else:
                val = beta_all[c, k] @ q64
            partition[b] = (
                np.log(val) + offk + corr_all[c, k] + logbn_all[c, k]
            )

    # --- gold path score (host, float64) ---
    maskf = mask.astype(np.float64)
    trans_tt = lt64[tags[:, :-1], tags[:, 1:]]
    emis = np.take_along_axis(
        feats64[:, :-1, :], tags[:, :-1, None].astype(np.int64), axis=2
    )[..., 0]
    scores = lt64[ROOT, tags[:, 0]]
    scores = scores + (trans_tt * maskf[:, 1:] + emis * maskf[:, :-1]).sum(axis=1)
    last_idx = (maskf.sum(axis=1) - 1.0).astype(np.int64)
    last_tags = np.take_along_axis(np.asarray(tags, np.int64), last_idx[:, None], axis=1)[:, 0]
    last_input = np.take_along_axis(feats64[:, -1, :], last_tags[:, None], axis=1)[:, 0]
    scores = scores + lt64[last_tags, END] + last_input * maskf[:, -1]

    return np.asarray((partition - scores).mean(), dtype=np.float32)
